# revision 11
# baseline (speedup 1.0000x reference)
"""Channel-attention kernel for Trainium2 (8 NeuronCores, batch-parallel).

Reference computation per batch b (feat (C, HW2), word_emb (N, D)):
    we0   = word_emb @ W_fc^T                 (N, HW2)
    S     = feat @ we0^T                      (C, N)   [b_fc shifts every logit
                                                        of a row equally -> the
                                                        softmax is invariant]
    A     = softmax(S, axis=-1)
    out   = A @ we0 + b_fc                    (C, HW2) [b_fc added on host]

v2 design (vs the v1 108.6us baseline; measured 65.5us, err 7.5e-3):
  - feat ships fp16-hi ONLY (half the input DMA); the score runs 2 fp16
    chains (we0hi + we0lo vs fthi) instead of 3 -> 16 score matmuls not 24.
  - output is stored fp16 (half the output DMA); host casts to fp32.
  - softmax normalization is folded into the O-phase: O = E^T @ we0 runs as
    f32r matmuls (full-rate for N>=256, HW-validated ~1.5e-4) with the
    UN-normalized E as stationary, per-c sums come from a tiny extra matmul
    against a ones column, and the PSUM->SBUF out-copy applies the 1/sums
    per-partition scale (ACT half / DVE half). Kills v1's (1,512) reciprocal
    row, the ones-broadcast matmul and the A=E*rb multiply.
  - we0 is transposed ONCE in fp32 (8 PE transposes, via bitcast of the f32r
    tile) and the fp16 hi/lo split happens straight from the transpose PSUM,
    instead of v1's n-partitioned split + 16 fp16 transposes.
  - wembT (word_emb^T hi/lo) and wfcT (W_fc^T hi/lo) are marshalled on host,
    killing v1's on-device wn splits + wembT transposes + wfcT setup
    transposes.
  Numerics (numpy-emulated on the real seed): scale-relative absmax ~7.8e-3
  vs the 2e-2 gate; measured on HW: 7.52e-3.

  Post-mortem of rejected variants (each measured slower on HW):
  - o_phase emitted before wt/score (73.8us / 71.4us): the O-phase DVE ops
    queue ahead of the wtlo sub on the in-order DVE queue and stall score.
  - merged 2-bank PSUM tiles + single per-batch out-DMA (71.4us): coarser
    PSUM rotation serializes O matmuls; the batched out-DMA lengthens the
    tail.
  - score as ONE chain, 8 matmuls (69.4us, err 1.34e-2): below this point
    the fixed head/tail (~25us) and HAM throttle dominate; removing PE work
    does not shorten the critical path but does burn error budget.

Device dataflow per batch (one NeuronCore handles B/8 = 4 batches):
    we0 psum    = 3 fp16-pair chains wembT^T @ wfcT      (12 mm, ~fp32-exact)
    we0         = ACT copies psum -> SBUF f32r           (O-phase moving)
    we0T psum   = 8 fp32 PE transposes of we0 (bitcast)
    wt hi       = ACT copy we0T psum -> fp16
    wt lo       = DVE sub (we0T psum - wt hi) -> fp16
    S^T         = wthi^T@FThi + wtlo^T@FThi              (16 mm into one PSUM)
    Eh          = exp(0.5*S^T - 48)                      (ACT; softmax-exact,
                                                          overflow-safe shift)
    E           = Eh*Eh -> f32r                          (DVE; = exp(S^T-96))
    per ct(4):  O0/O1 = E-slice^T @ we0 halves (f32r), sums = E-slice^T @ ones
                rr = 1/sums (DVE, (128,1)); out fp16 = psum * rr (ACT + DVE),
                per-ct out DMA on the scalar queue
"""

import numpy as np

import concourse.bass as bass
import concourse.mybir as mybir
import concourse.tile as tile
from concourse import bacc
from concourse.bass import ds, ts
from concourse.bass_utils import run_bass_kernel_spmd
from concourse.masks import make_identity

B, C, HW2 = 32, 512, 1024
N_WORDS, WORD_DIM = 77, 256
H = W = 32
N_CORES = 8
BPC = B // N_CORES  # batches per core
NP = 80  # N_WORDS padded to a multiple of 16

FP32 = mybir.dt.float32
FP16 = mybir.dt.float16
F32R = mybir.dt.float32r
AF = mybir.ActivationFunctionType

EXP_SCALE = 0.5
EXP_BIAS = -48.0  # exp(0.5*s - 48)^2 == exp(s - 96)

LAST_RESULT = None  # BassKernelResults of the most recent run (for test.py)


def _body(nc, tc, ftp_d, wemb_d, wfc_d, out_d):
    from contextlib import ExitStack

    with ExitStack() as ctx:
        const = ctx.enter_context(tc.tile_pool(name="const", bufs=1))
        big = ctx.enter_context(tc.tile_pool(name="big", bufs=2))
        med = ctx.enter_context(tc.tile_pool(name="med", bufs=2))
        outp = ctx.enter_context(tc.tile_pool(name="outp", bufs=2))
        mm_ps = ctx.enter_context(tc.tile_pool(name="mm_ps", bufs=4, space="PSUM"))
        tp_ps = ctx.enter_context(tc.tile_pool(name="tp_ps", bufs=2, space="PSUM"))
        sc_ps = ctx.enter_context(tc.tile_pool(name="sc_ps", bufs=1, space="PSUM"))
        su_ps = ctx.enter_context(tc.tile_pool(name="su_ps", bufs=1, space="PSUM"))

        ident = const.tile([128, 128], FP32)
        make_identity(nc, ident[:])
        ones_f = const.tile([128, 8], FP32)
        nc.gpsimd.memset(ones_f[:], 1.0)
        ones = const.tile([128, 8], F32R)
        nc.vector.tensor_copy(ones[:], ones_f[:])
        ebias = const.tile([128, 1], FP32)
        nc.gpsimd.memset(ebias[:], EXP_BIAS)

        # W_fc^T hi|lo packed, host-marshalled: (256, 2048) -> (128, 2, 2048)
        vt = const.tile([128, 2, 2048], FP16)

        def load(b, st, ft_eng=None):
            # wembT packed (256, 160) = [hi(80) | lo(80)] -> (128, 2, 160)
            wembT = st["wembT"] = med.tile(
                [128, 2, 160], FP16, tag="wembT", name="wembT"
            )
            nc.sync.dma_start(wembT[:], wemb_d[b].rearrange("(c p) x -> p c x", p=128))
            # FT hi (hw-partitioned): (1024, 512) -> (128, 8, 512)
            ft = st["ft"] = big.tile([128, 8, 512], FP16, tag="ft", name="ft")
            (ft_eng or nc.sync).dma_start(
                ft[:], ftp_d[b].rearrange("(t p) x -> p t x", p=128)
            )

        def we0_phase(st):
            # we0 = word_emb @ W_fc^T (77, 1024): 3 fp16-pair chains
            wembT = st["wembT"]
            whi = (wembT[:, 0, :NP], wembT[:, 1, :NP])
            wlo = (wembT[:, 0, NP:], wembT[:, 1, NP:])
            we0 = st["we0"] = med.tile([128, 1024], F32R, tag="we0", name="we0")
            for half in range(2):
                ps = mm_ps.tile([128, 512], FP32, tag="mm")
                sl = ds(half * 512, 512)
                mms = []
                for dc in range(2):
                    mms.append((whi[dc], vt[:, dc, ds(half * 512, 512)]))
                for dc in range(2):
                    mms.append((whi[dc], vt[:, dc, ds(1024 + half * 512, 512)]))
                    mms.append((wlo[dc], vt[:, dc, ds(half * 512, 512)]))
                for j, (lhsT, rhs) in enumerate(mms):
                    nc.tensor.matmul(
                        ps[:NP, :], lhsT, rhs, start=(j == 0), stop=(j == 5)
                    )
                nc.scalar.copy(we0[:N_WORDS, sl], ps[:N_WORDS, :])

        def wt_phase(st):
            # we0^T via 8 fp32 PE transposes; split hi/lo straight from PSUM
            we0 = st["we0"][:].bitcast(FP32)
            wthi = st["wthi"] = med.tile([128, 8, NP], FP16, tag="wthi", name="wthi")
            wtlo = st["wtlo"] = med.tile([128, 8, NP], FP16, tag="wtlo", name="wtlo")
            for g in range(2):
                tps = tp_ps.tile([128, 4, NP], FP32, tag="tp")
                for j in range(4):
                    nc.tensor.matmul(
                        tps[:, j, :N_WORDS],
                        we0[:N_WORDS, ts(g * 4 + j, 128)],
                        ident[:N_WORDS, :N_WORDS],
                        is_transpose=True,
                        start=(j == 0),
                        stop=(j == 3),
                    )
                gs = ds(g * 4, 4)
                nc.scalar.copy(wthi[:, gs, :N_WORDS], tps[:, :, :N_WORDS])
                nc.vector.tensor_sub(
                    wtlo[:, gs, :N_WORDS], tps[:, :, :N_WORDS], wthi[:, gs, :N_WORDS]
                )

        def score(st):
            # S^T = wthi^T @ FThi + wtlo^T @ FThi  (77, 512), 16 matmuls
            ft = st["ft"]
            sps = st["sps"] = sc_ps.tile([128, 512], FP32, tag="sc", name="sps")
            i = 0
            for src in (st["wthi"], st["wtlo"]):
                for kt in range(8):
                    nc.tensor.matmul(
                        sps[:NP, :],
                        src[:, kt, :],
                        ft[:, kt, :],
                        start=(i == 0),
                        stop=(i == 15),
                    )
                    i += 1

        def soft(st):
            # E = exp(S^T - 96) via exp(0.5*s - 48)^2, kept f32r un-normalized
            sps = st["sps"]
            ehalf = med.tile([128, 512], FP32, tag="ehalf")
            nc.scalar.activation(
                ehalf[:N_WORDS, :],
                sps[:N_WORDS, :],
                AF.Exp,
                bias=ebias[:N_WORDS, :],
                scale=EXP_SCALE,
            )
            eT = st["eT"] = med.tile([128, 512], F32R, tag="eT", name="eT")
            nc.vector.tensor_mul(eT[:N_WORDS, :], ehalf[:N_WORDS, :], ehalf[:N_WORDS, :])

        def o_phase(st, b):
            # O = E-slice^T @ we0 (f32r) + ones-column sums; normalize on copy
            eT, we0 = st["eT"], st["we0"]
            rr = med.tile([128, 4], FP32, tag="rr")
            for ct in range(4):
                e_sl = eT[:N_WORDS, ts(ct, 128)]
                ops0 = mm_ps.tile([128, 512], FP32, tag="mm")
                nc.tensor.matmul(ops0[:], e_sl, we0[:N_WORDS, :512])
                ops1 = mm_ps.tile([128, 512], FP32, tag="mm")
                nc.tensor.matmul(ops1[:], e_sl, we0[:N_WORDS, 512:])
                su = su_ps.tile([128, 8], FP32, tag="su")
                nc.tensor.matmul(su[:], e_sl, ones[:N_WORDS, :])
                rc = rr[:, ct : ct + 1]
                nc.vector.reciprocal(rc, su[:, 0:1])
                ob = outp.tile([128, 1024], FP16, tag="ob")
                nc.scalar.mul(ob[:, :512], ops0[:], rc)
                nc.vector.tensor_scalar_mul(ob[:, 512:], ops1[:], rc)
                nc.scalar.dma_start(out_d[b, ts(ct, 128), :], ob[:])

        # software pipeline: batch b's O phase is emitted behind batch b+1's
        # score so the (in-order) PE queue always has independent work while
        # b's softmax chain runs on ACT/DVE.
        states = {b: {} for b in range(BPC)}
        # head: wembT0 + wfc-hi on the sync queue, ft0 + wfc-lo on the scalar
        # queue, so batch 0's we0 starts as soon as wfc-hi lands.
        load(0, states[0], ft_eng=nc.scalar)
        nc.sync.dma_start(
            vt[:, :, :1024], wfc_d[:, :1024].rearrange("(c p) x -> p c x", p=128)
        )
        nc.scalar.dma_start(
            vt[:, :, 1024:], wfc_d[:, 1024:].rearrange("(c p) x -> p c x", p=128)
        )
        we0_phase(states[0])
        wt_phase(states[0])
        load(1, states[1])
        score(states[0])
        soft(states[0])
        for b in range(1, BPC):
            we0_phase(states[b])
            wt_phase(states[b])
            if b + 1 < BPC:
                load(b + 1, states[b + 1])
            score(states[b])
            soft(states[b])
            o_phase(states[b - 1], b - 1)
            del states[b - 1]
        o_phase(states[BPC - 1], BPC - 1)


def _build():
    nc = bacc.Bacc(
        "TRN2",
        target_bir_lowering=False,
        debug=False,
        enable_asserts=False,
        num_devices=N_CORES,
    )
    ftp_d = nc.declare_dram_parameter("ftp", [BPC, HW2, C], FP16, isOutput=False)
    wemb_d = nc.declare_dram_parameter(
        "wemb", [BPC, WORD_DIM, 2 * NP], FP16, isOutput=False
    )
    wfc_d = nc.declare_dram_parameter("wfc", [WORD_DIM, 2048], FP16, isOutput=False)
    out_d = nc.declare_dram_parameter("out", [BPC, C, HW2], FP16, isOutput=True)
    with tile.TileContext(nc) as tc:
        _body(nc, tc, ftp_d, wemb_d, wfc_d, out_d)
    nc.finalize()
    return nc


_CACHE = {}


def kernel(feat, word_emb, W_fc, b_fc, **run_kwargs):
    global LAST_RESULT
    feat = np.asarray(feat, dtype=np.float32).reshape(B, C, HW2)
    word_emb = np.asarray(word_emb, dtype=np.float32)
    W_fc = np.asarray(W_fc, dtype=np.float32)
    b_fc = np.asarray(b_fc, dtype=np.float32)

    # host marshalling (layout/dtype only):
    # feat -> fp16 hi of feat^T, (B, HW2, C)
    ftp = np.ascontiguousarray(feat.transpose(0, 2, 1)).astype(np.float16)
    # word_emb^T hi/lo packed (B, 256, 160) = [hi(77->80) | lo(77->80)]
    wembT = np.ascontiguousarray(word_emb.transpose(0, 2, 1))  # (B, 256, 77)
    whi = wembT.astype(np.float16)
    wlo = (wembT - whi.astype(np.float32)).astype(np.float16)
    wembp = np.zeros((B, WORD_DIM, 2 * NP), dtype=np.float16)
    wembp[:, :, :N_WORDS] = whi
    wembp[:, :, NP : NP + N_WORDS] = wlo
    # W_fc^T hi/lo packed (256, 2048) = [hi(1024) | lo(1024)]
    vT = np.ascontiguousarray(W_fc.T)  # (256, 1024)
    vhi = vT.astype(np.float16)
    vlo = (vT - vhi.astype(np.float32)).astype(np.float16)
    wfcp = np.concatenate([vhi, vlo], axis=1)  # (256, 2048)

    if "nc" not in _CACHE:
        _CACHE["nc"] = _build()
    nc = _CACHE["nc"]

    in_maps = [
        {
            "ftp": ftp[i * BPC : (i + 1) * BPC],
            "wemb": wembp[i * BPC : (i + 1) * BPC],
            "wfc": wfcp,
        }
        for i in range(N_CORES)
    ]
    res = run_bass_kernel_spmd(nc, in_maps, list(range(N_CORES)), **run_kwargs)
    LAST_RESULT = res
    out = np.concatenate([res.results[i]["out"] for i in range(N_CORES)], axis=0)
    # b_fc shifts all logits of a softmax row equally (no effect on A) and
    # adds linearly to the output: out = A @ we0 + b_fc. Exact identity.
    out = out.astype(np.float32) + b_fc.reshape(1, 1, HW2)
    return out.reshape(B, C, H, W).astype(np.float32)


# revision 12
# speedup vs baseline: 1.0618x; 1.0618x over previous
"""Channel-attention kernel for Trainium2 (8 NeuronCores, batch-parallel).

Reference computation per batch b (feat (C, HW2), word_emb (N, D)):
    we0   = word_emb @ W_fc^T                 (N, HW2)
    S     = feat @ we0^T                      (C, N)   [b_fc shifts every logit
                                                        of a row equally -> the
                                                        softmax is invariant]
    A     = softmax(S, axis=-1)
    out   = A @ we0 + b_fc                    (C, HW2) [b_fc added on host]

v2 design (vs the v1 108.6us baseline; measured 65.5us, err 7.5e-3):
  - feat ships fp16-hi ONLY (half the input DMA); the score runs 2 fp16
    chains (we0hi + we0lo vs fthi) instead of 3 -> 16 score matmuls not 24.
  - output is stored fp16 (half the output DMA); host casts to fp32.
  - softmax normalization is folded into the O-phase: O = E^T @ we0 runs as
    f32r matmuls (full-rate for N>=256, HW-validated ~1.5e-4) with the
    UN-normalized E as stationary, per-c sums come from a tiny extra matmul
    against a ones column, and the PSUM->SBUF out-copy applies the 1/sums
    per-partition scale (ACT half / DVE half). Kills v1's (1,512) reciprocal
    row, the ones-broadcast matmul and the A=E*rb multiply.
  - we0 is transposed ONCE in fp32 (8 PE transposes, via bitcast of the f32r
    tile) and the fp16 hi/lo split happens straight from the transpose PSUM,
    instead of v1's n-partitioned split + 16 fp16 transposes.
  - wembT (word_emb^T hi/lo) and wfcT (W_fc^T hi/lo) are marshalled on host,
    killing v1's on-device wn splits + wembT transposes + wfcT setup
    transposes.
  Numerics (numpy-emulated on the real seed): scale-relative absmax ~7.8e-3
  vs the 2e-2 gate; measured on HW: 7.52e-3.

  Post-mortem of rejected variants (each measured slower on HW):
  - o_phase emitted before wt/score (73.8us / 71.4us): the O-phase DVE ops
    queue ahead of the wtlo sub on the in-order DVE queue and stall score.
  - merged 2-bank PSUM tiles + single per-batch out-DMA (71.4us): coarser
    PSUM rotation serializes O matmuls; the batched out-DMA lengthens the
    tail.
  - score as ONE chain, 8 matmuls (69.4us, err 1.34e-2): below this point
    the fixed head/tail (~25us) and HAM throttle dominate; removing PE work
    does not shorten the critical path but does burn error budget.

Device dataflow per batch (one NeuronCore handles B/8 = 4 batches):
    we0 psum    = 3 fp16-pair chains wembT^T @ wfcT      (12 mm, ~fp32-exact)
    we0         = ACT copies psum -> SBUF f32r           (O-phase moving)
    we0T psum   = 8 fp32 PE transposes of we0 (bitcast)
    wt hi       = ACT copy we0T psum -> fp16
    wt lo       = DVE sub (we0T psum - wt hi) -> fp16
    S^T         = wthi^T@FThi + wtlo^T@FThi              (16 mm into one PSUM)
    Eh          = exp(0.5*S^T - 48)                      (ACT; softmax-exact,
                                                          overflow-safe shift)
    E           = Eh*Eh -> f32r                          (DVE; = exp(S^T-96))
    per ct(4):  O0/O1 = E-slice^T @ we0 halves (f32r), sums = E-slice^T @ ones
                rr = 1/sums (DVE, (128,1)); out fp16 = psum * rr (ACT + DVE),
                per-ct out DMA on the scalar queue
"""

import numpy as np

import concourse.bass as bass
import concourse.mybir as mybir
import concourse.tile as tile
from concourse import bacc
from concourse.bass import ds, ts
from concourse.bass_utils import run_bass_kernel_spmd
from concourse.masks import make_identity

B, C, HW2 = 32, 512, 1024
N_WORDS, WORD_DIM = 77, 256
H = W = 32
N_CORES = 8
BPC = B // N_CORES  # batches per core
NP = 80  # N_WORDS padded to a multiple of 16

FP32 = mybir.dt.float32
FP16 = mybir.dt.float16
F32R = mybir.dt.float32r
AF = mybir.ActivationFunctionType

EXP_SCALE = 0.5
EXP_BIAS = -48.0  # exp(0.5*s - 48)^2 == exp(s - 96)

LAST_RESULT = None  # BassKernelResults of the most recent run (for test.py)


def _body(nc, tc, ftp_d, wemb_d, wfc_d, out_d):
    from contextlib import ExitStack

    with ExitStack() as ctx:
        const = ctx.enter_context(tc.tile_pool(name="const", bufs=1))
        big = ctx.enter_context(tc.tile_pool(name="big", bufs=2))
        med = ctx.enter_context(tc.tile_pool(name="med", bufs=2))
        outp = ctx.enter_context(tc.tile_pool(name="outp", bufs=2))
        mm_ps = ctx.enter_context(tc.tile_pool(name="mm_ps", bufs=4, space="PSUM"))
        tp_ps = ctx.enter_context(tc.tile_pool(name="tp_ps", bufs=2, space="PSUM"))
        sc_ps = ctx.enter_context(tc.tile_pool(name="sc_ps", bufs=1, space="PSUM"))
        su_ps = ctx.enter_context(tc.tile_pool(name="su_ps", bufs=1, space="PSUM"))

        ident = const.tile([128, 128], FP32)
        make_identity(nc, ident[:])
        ones_f = const.tile([128, 8], FP32)
        nc.gpsimd.memset(ones_f[:], 1.0)
        ones = const.tile([128, 8], F32R)
        nc.vector.tensor_copy(ones[:], ones_f[:])
        ebias = const.tile([128, 1], FP32)
        nc.gpsimd.memset(ebias[:], EXP_BIAS)

        # W_fc^T hi|lo packed, host-marshalled: (256, 2048) -> (128, 2, 2048)
        vt = const.tile([128, 2, 2048], FP16)

        def load(b, st, ft_eng=None):
            # wembT packed (256, 160) = [hi(80) | lo(80)] -> (128, 2, 160)
            wembT = st["wembT"] = med.tile(
                [128, 2, 160], FP16, tag="wembT", name="wembT"
            )
            nc.sync.dma_start(wembT[:], wemb_d[b].rearrange("(c p) x -> p c x", p=128))
            # FT hi (hw-partitioned): (1024, 512) -> (128, 8, 512)
            ft = st["ft"] = big.tile([128, 8, 512], FP16, tag="ft", name="ft")
            (ft_eng or nc.sync).dma_start(
                ft[:], ftp_d[b].rearrange("(t p) x -> p t x", p=128)
            )

        def we0_phase(st):
            # we0 = word_emb @ W_fc^T (77, 1024): 3 fp16-pair chains
            wembT = st["wembT"]
            whi = (wembT[:, 0, :NP], wembT[:, 1, :NP])
            wlo = (wembT[:, 0, NP:], wembT[:, 1, NP:])
            we0 = st["we0"] = med.tile([128, 1024], F32R, tag="we0", name="we0")
            for half in range(2):
                ps = mm_ps.tile([128, 512], FP32, tag="mm")
                sl = ds(half * 512, 512)
                mms = []
                for dc in range(2):
                    vhi_sl = vt[:, dc, ds(half * 512, 512)]
                    vlo_sl = vt[:, dc, ds(1024 + half * 512, 512)]
                    mms += [(whi[dc], vhi_sl), (whi[dc], vlo_sl), (wlo[dc], vhi_sl)]
                for j, (lhsT, rhs) in enumerate(mms):
                    nc.tensor.matmul(
                        ps[:NP, :], lhsT, rhs, start=(j == 0), stop=(j == 5)
                    )
                nc.scalar.copy(we0[:N_WORDS, sl], ps[:N_WORDS, :])

        def wt_phase(st):
            # we0^T via 8 fp32 PE transposes; split hi/lo straight from PSUM
            we0 = st["we0"][:].bitcast(FP32)
            wthi = st["wthi"] = med.tile([128, 8, NP], FP16, tag="wthi", name="wthi")
            wtlo = st["wtlo"] = med.tile([128, 8, NP], FP16, tag="wtlo", name="wtlo")
            for g in range(2):
                tps = tp_ps.tile([128, 4, NP], FP32, tag="tp")
                for j in range(4):
                    nc.tensor.matmul(
                        tps[:, j, :N_WORDS],
                        we0[:N_WORDS, ts(g * 4 + j, 128)],
                        ident[:N_WORDS, :N_WORDS],
                        is_transpose=True,
                        start=(j == 0),
                        stop=(j == 3),
                    )
                gs = ds(g * 4, 4)
                nc.scalar.copy(wthi[:, gs, :N_WORDS], tps[:, :, :N_WORDS])
                nc.vector.tensor_sub(
                    wtlo[:, gs, :N_WORDS], tps[:, :, :N_WORDS], wthi[:, gs, :N_WORDS]
                )

        def score(st):
            # S^T = wthi^T @ FThi + wtlo^T @ FThi  (77, 512), 16 matmuls
            ft = st["ft"]
            sps = st["sps"] = sc_ps.tile([128, 512], FP32, tag="sc", name="sps")
            i = 0
            for src in (st["wthi"], st["wtlo"]):
                for kt in range(8):
                    nc.tensor.matmul(
                        sps[:NP, :],
                        src[:, kt, :],
                        ft[:, kt, :],
                        start=(i == 0),
                        stop=(i == 15),
                    )
                    i += 1

        def soft(st):
            # E = exp(S^T - 96) via exp(0.5*s - 48)^2, kept f32r un-normalized
            sps = st["sps"]
            ehalf = med.tile([128, 512], FP32, tag="ehalf")
            nc.scalar.activation(
                ehalf[:N_WORDS, :],
                sps[:N_WORDS, :],
                AF.Exp,
                bias=ebias[:N_WORDS, :],
                scale=EXP_SCALE,
            )
            eT = st["eT"] = med.tile([128, 512], F32R, tag="eT", name="eT")
            nc.vector.tensor_mul(eT[:N_WORDS, :], ehalf[:N_WORDS, :], ehalf[:N_WORDS, :])

        def o_phase(st, b):
            # O = E-slice^T @ we0 (f32r) + ones-column sums; normalize on copy
            eT, we0 = st["eT"], st["we0"]
            rr = med.tile([128, 4], FP32, tag="rr")
            for ct in range(4):
                e_sl = eT[:N_WORDS, ts(ct, 128)]
                ops0 = mm_ps.tile([128, 512], FP32, tag="mm")
                nc.tensor.matmul(ops0[:], e_sl, we0[:N_WORDS, :512])
                ops1 = mm_ps.tile([128, 512], FP32, tag="mm")
                nc.tensor.matmul(ops1[:], e_sl, we0[:N_WORDS, 512:])
                su = su_ps.tile([128, 8], FP32, tag="su")
                nc.tensor.matmul(su[:], e_sl, ones[:N_WORDS, :])
                rc = rr[:, ct : ct + 1]
                nc.vector.reciprocal(rc, su[:, 0:1])
                ob = outp.tile([128, 1024], FP16, tag="ob")
                nc.scalar.mul(ob[:, :512], ops0[:], rc)
                nc.vector.tensor_scalar_mul(ob[:, 512:], ops1[:], rc)
                nc.scalar.dma_start(out_d[b, ts(ct, 128), :], ob[:])

        # software pipeline: batch b's O phase is emitted behind batch b+1's
        # score so the (in-order) PE queue always has independent work while
        # b's softmax chain runs on ACT/DVE.
        states = {b: {} for b in range(BPC)}
        load(0, states[0])
        nc.sync.dma_start(vt[:], wfc_d.rearrange("(c p) x -> p c x", p=128))
        we0_phase(states[0])
        wt_phase(states[0])
        load(1, states[1])
        score(states[0])
        soft(states[0])
        for b in range(1, BPC):
            we0_phase(states[b])
            wt_phase(states[b])
            if b + 1 < BPC:
                load(b + 1, states[b + 1])
            score(states[b])
            o_phase(states[b - 1], b - 1)
            soft(states[b])
            del states[b - 1]
        o_phase(states[BPC - 1], BPC - 1)


def _build():
    nc = bacc.Bacc(
        "TRN2",
        target_bir_lowering=False,
        debug=False,
        enable_asserts=False,
        num_devices=N_CORES,
    )
    ftp_d = nc.declare_dram_parameter("ftp", [BPC, HW2, C], FP16, isOutput=False)
    wemb_d = nc.declare_dram_parameter(
        "wemb", [BPC, WORD_DIM, 2 * NP], FP16, isOutput=False
    )
    wfc_d = nc.declare_dram_parameter("wfc", [WORD_DIM, 2048], FP16, isOutput=False)
    out_d = nc.declare_dram_parameter("out", [BPC, C, HW2], FP16, isOutput=True)
    with tile.TileContext(nc) as tc:
        _body(nc, tc, ftp_d, wemb_d, wfc_d, out_d)
    nc.finalize()
    return nc


_CACHE = {}


def kernel(feat, word_emb, W_fc, b_fc, **run_kwargs):
    global LAST_RESULT
    feat = np.asarray(feat, dtype=np.float32).reshape(B, C, HW2)
    word_emb = np.asarray(word_emb, dtype=np.float32)
    W_fc = np.asarray(W_fc, dtype=np.float32)
    b_fc = np.asarray(b_fc, dtype=np.float32)

    # host marshalling (layout/dtype only):
    # feat -> fp16 hi of feat^T, (B, HW2, C)
    ftp = np.ascontiguousarray(feat.transpose(0, 2, 1)).astype(np.float16)
    # word_emb^T hi/lo packed (B, 256, 160) = [hi(77->80) | lo(77->80)]
    wembT = np.ascontiguousarray(word_emb.transpose(0, 2, 1))  # (B, 256, 77)
    whi = wembT.astype(np.float16)
    wlo = (wembT - whi.astype(np.float32)).astype(np.float16)
    wembp = np.zeros((B, WORD_DIM, 2 * NP), dtype=np.float16)
    wembp[:, :, :N_WORDS] = whi
    wembp[:, :, NP : NP + N_WORDS] = wlo
    # W_fc^T hi/lo packed (256, 2048) = [hi(1024) | lo(1024)]
    vT = np.ascontiguousarray(W_fc.T)  # (256, 1024)
    vhi = vT.astype(np.float16)
    vlo = (vT - vhi.astype(np.float32)).astype(np.float16)
    wfcp = np.concatenate([vhi, vlo], axis=1)  # (256, 2048)

    if "nc" not in _CACHE:
        _CACHE["nc"] = _build()
    nc = _CACHE["nc"]

    in_maps = [
        {
            "ftp": ftp[i * BPC : (i + 1) * BPC],
            "wemb": wembp[i * BPC : (i + 1) * BPC],
            "wfc": wfcp,
        }
        for i in range(N_CORES)
    ]
    res = run_bass_kernel_spmd(nc, in_maps, list(range(N_CORES)), **run_kwargs)
    LAST_RESULT = res
    out = np.concatenate([res.results[i]["out"] for i in range(N_CORES)], axis=0)
    # b_fc shifts all logits of a softmax row equally (no effect on A) and
    # adds linearly to the output: out = A @ we0 + b_fc. Exact identity.
    out = out.astype(np.float32) + b_fc.reshape(1, 1, HW2)
    return out.reshape(B, C, H, W).astype(np.float32)
